# revision 1
# baseline (speedup 1.0000x reference)
"""Swin-style windowed local self-attention (LN -> QKV -> 7x7 window MHA
with relative position bias -> proj) on 8 Trainium2 NeuronCores.

Sharding: pure data parallel over B*T (24 images -> 3 per core).

Device-side design (per core: 9408 tokens = 192 windows = 96 window-pairs):
  - host folds ln_g + attention scale into the QKV weights, ships bf16
    weights; window reordering of x / output happens on host (numpy).
  - LN on [98,384] token tiles (bn_stats/bn_aggr), normalized output cast
    to bf16, padded to 112 rows for the DMA-xbar transpose.
  - x^T (feature-major) built with DMA transpose; QKV q/k computed
    feature-major (weights stationary, token chunks N<=512), v computed
    token-major per window with PSUM column tiling (w0 rows 0-48,
    w1 rows 64-112 -> concurrent PE sub-tiles).
  - scores are computed transposed (S^T = [k,q]) so that after bias+exp
    the [k,q] tile IS the lhsT of the P@V matmul -- no transpose of the
    softmax matrix is ever needed.  Relative-position bias is accumulated
    into the score PSUM by an extra PE matmul against a duplicated
    identity.  exp runs on the scalar engine straight out of PSUM.
  - softmax normalization: V is augmented with a ones column, so the P@V
    matmul also produces the row sums; a reciprocal + broadcast-multiply
    at PSUM evacuation normalizes (per-partition = per-query-token).
  - attention output is transposed back to feature-major by DMA transpose
    and hits the proj matmul (token-major out), bias-added and stored.
"""

import sys

if "/opt/trn_rl_repo" not in sys.path:
    sys.path.insert(0, "/opt/trn_rl_repo")

import numpy as np
import ml_dtypes

import concourse.bacc as bacc
import concourse.bass as bass
import concourse.tile as tile
import concourse.mybir as mybir
from concourse.bass_utils import run_bass_kernel_spmd

F32 = mybir.dt.float32
BF16 = mybir.dt.bfloat16

N_CORES = 8
B, T, H, W, D = 4, 6, 56, 56, 384
WSZ = 7
NH = 12
HD = D // NH            # 32
N = WSZ * WSZ           # 49 tokens / window
NW_IMG = (H // WSZ) * (W // WSZ)   # 64 windows / image
IMGS_CORE = (B * T) // N_CORES     # 3
TOK_CORE = IMGS_CORE * H * W       # 9408
NPAIR = TOK_CORE // (2 * N)        # 96 window pairs / core
EPS = 1e-5

# group = contiguous run of window pairs whose x^T / q / k stay in SBUF
N_GROUPS = 4
PAIRS_G = NPAIR // N_GROUPS        # 24
TOK_G = PAIRS_G * 2 * N            # 2352
QK_CHUNK = 512


def _rel_index(w):
    coords = np.stack(np.meshgrid(np.arange(w), np.arange(w), indexing="ij")).reshape(2, -1)
    rel = coords[:, :, None] - coords[:, None, :]
    return (rel[0] + w - 1) * (2 * w - 1) + (rel[1] + w - 1)


def build_program(n_groups=N_GROUPS, debug_dump=False, repeat=1):
    nc = bacc.Bacc("TRN2", target_bir_lowering=False, debug=False, num_devices=N_CORES)

    x_d = nc.dram_tensor("x", [TOK_CORE, D], F32, kind="ExternalInput")
    qkw_d = nc.dram_tensor("qkw", [D, 2 * D], BF16, kind="ExternalInput")
    vw_d = nc.dram_tensor("vw", [D, D], BF16, kind="ExternalInput")
    pw_d = nc.dram_tensor("pw", [D, D], BF16, kind="ExternalInput")
    cqk_d = nc.dram_tensor("cqk", [2 * D], F32, kind="ExternalInput")
    cv_d = nc.dram_tensor("cv", [D], F32, kind="ExternalInput")
    pb_d = nc.dram_tensor("pb", [D], F32, kind="ExternalInput")
    bmm_d = nc.dram_tensor("bmm", [128, NH * N], BF16, kind="ExternalInput")
    idup_d = nc.dram_tensor("idup", [128, 64], BF16, kind="ExternalInput")
    y_d = nc.dram_tensor("y", [TOK_CORE, D], F32, kind="ExternalOutput")
    if debug_dump:
        xT_o = nc.dram_tensor("xT_o", [3, 128, PAIRS_G * 112], BF16, kind="ExternalOutput")
        qk_o = nc.dram_tensor("qk_o", [6, 128, TOK_G], BF16, kind="ExternalOutput")
        av_o = nc.dram_tensor("av_o", [PAIRS_G, 128, NH * (HD + 1)], BF16, kind="ExternalOutput")
        pt_o = nc.dram_tensor("pt_o", [PAIRS_G, 128, NH * N], BF16, kind="ExternalOutput")

    from contextlib import ExitStack
    with tile.TileContext(nc) as tc, ExitStack() as ctx:
        const = ctx.enter_context(tc.tile_pool(name="const", bufs=1))
        grp = ctx.enter_context(tc.tile_pool(name="grp", bufs=2))
        work = ctx.enter_context(tc.tile_pool(name="work", bufs=3))
        small = ctx.enter_context(tc.tile_pool(name="small", bufs=4))
        ps_qk = ctx.enter_context(tc.tile_pool(name="ps_qk", bufs=2, space="PSUM"))
        ps_v = ctx.enter_context(tc.tile_pool(name="ps_v", bufs=1, space="PSUM"))
        ps_s = ctx.enter_context(tc.tile_pool(name="ps_s", bufs=2, space="PSUM"))
        ps_o = ctx.enter_context(tc.tile_pool(name="ps_o", bufs=2, space="PSUM"))
        ps_p = ctx.enter_context(tc.tile_pool(name="ps_p", bufs=1, space="PSUM"))

        # ---- resident constants -------------------------------------------------
        qkw_sb = [const.tile([128, 2 * D], BF16, name=f"qkw{k}", tag=f"qkw{k}") for k in range(3)]
        vw_sb = [const.tile([128, D], BF16, name=f"vw{k}", tag=f"vw{k}") for k in range(3)]
        pw_sb = [const.tile([128, D], BF16, name=f"pw{k}", tag=f"pw{k}") for k in range(3)]
        for k in range(3):
            nc.sync.dma_start(out=qkw_sb[k][:], in_=qkw_d[128 * k:128 * (k + 1), :])
            nc.sync.dma_start(out=vw_sb[k][:], in_=vw_d[128 * k:128 * (k + 1), :])
            nc.sync.dma_start(out=pw_sb[k][:], in_=pw_d[128 * k:128 * (k + 1), :])
        cqk_sb = [const.tile([128, 1], F32, name=f"cqk{m}", tag=f"cqk{m}") for m in range(6)]
        for m in range(6):
            nc.sync.dma_start(out=cqk_sb[m][:], in_=cqk_d[128 * m:128 * (m + 1)])
        def bcast128(dram_ap):
            return bass.AP(tensor=dram_ap.tensor, offset=dram_ap.offset,
                           ap=[[0, 128], *dram_ap.ap])

        cv_sb = const.tile([128, D], F32, name="cv", tag="cv")
        nc.sync.dma_start(out=cv_sb[:], in_=bcast128(cv_d[:]))
        pb_sb = const.tile([128, D], F32, name="pb", tag="pb")
        nc.sync.dma_start(out=pb_sb[:], in_=bcast128(pb_d[:]))
        bmm_sb = const.tile([128, NH * N], BF16, name="bmm", tag="bmm")
        nc.sync.dma_start(out=bmm_sb[:], in_=bmm_d[:])
        idup_sb = const.tile([128, 64], BF16, name="idup", tag="idup")
        nc.sync.dma_start(out=idup_sb[:], in_=idup_d[:])
        eps_sb = const.tile([128, 1], F32, name="eps", tag="eps")
        nc.vector.memset(eps_sb[:], EPS)

        # persistent rotating tiles whose pad regions are initialized once
        NROT = 3
        xn_rot = [const.tile([128, D], BF16, name=f"xn{i}", tag=f"xn{i}") for i in range(NROT)]
        for t in xn_rot:
            # zero the transpose pad rows 98-111 once (96-aligned start; rows
            # 96-97 are rewritten by every normalize before any transpose reads)
            nc.gpsimd.memset(t[96:112, :], 0.0)
        av_rot = [const.tile([128, NH, HD + 1], BF16, name=f"av{i}", tag=f"av{i}") for i in range(NROT)]
        for t in av_rot:
            nc.gpsimd.memset(t[:, :, HD:HD + 1], 1.0)

        rep_ctx = tc.For_i(0, repeat, 1) if repeat > 1 else None
        if rep_ctx is not None:
            rep_ctx.__enter__()
        for g in range(n_groups):
            T0 = g * TOK_G
            xT = [grp.tile([128, PAIRS_G * 112], BF16, name=f"xT{k}", tag=f"xT{k}") for k in range(3)]
            qk = [grp.tile([128, TOK_G], BF16, name=f"qk{m}", tag=f"qk{m}") for m in range(6)]

            # ---- phase A: LN + transpose ---------------------------------------
            for p in range(PAIRS_G):
                r0 = T0 + 98 * p
                x_t = work.tile([128, D], F32, name="x", tag="x")
                nc.sync.dma_start(out=x_t[0:98, :], in_=x_d[r0:r0 + 98, :])
                stats = small.tile([128, 6], F32, name="stats", tag="stats")
                nc.vector.bn_stats(out=stats[0:98, :], in_=x_t[0:98, :])
                mv = small.tile([128, 2], F32, name="mv", tag="mv")
                nc.vector.bn_aggr(out=mv[0:98, :], in_=stats[0:98, :])
                nc.scalar.activation(
                    out=mv[0:98, 1:2], in_=mv[0:98, 1:2],
                    func=mybir.ActivationFunctionType.Sqrt,
                    bias=eps_sb[0:98, :], scale=1.0,
                )
                nc.vector.reciprocal(out=mv[0:98, 1:2], in_=mv[0:98, 1:2])
                xn = xn_rot[p % NROT]
                nc.vector.tensor_scalar(
                    out=xn[0:98, :], in0=x_t[0:98, :],
                    scalar1=mv[0:98, 0:1], scalar2=mv[0:98, 1:2],
                    op0=mybir.AluOpType.subtract, op1=mybir.AluOpType.mult,
                )
                for k in range(3):
                    # xbar transpose: out column offset must be 16-aligned,
                    # hence the 112-wide per-pair slots
                    nc.sync.dma_start(
                        out=xT[k][:, 112 * p:112 * p + 112],
                        in_=xn[0:112, 128 * k:128 * (k + 1)],
                        transpose=True,
                    )

            # ---- phase B: q/k projections (feature-major) ----------------------
            # rhs is a strided view skipping the 14 pad cols of each 112-slot
            PCH = 5
            for pc in range(0, PAIRS_G, PCH):
                np_ = min(PCH, PAIRS_G - pc)
                w = 98 * np_
                for m in range(6):
                    pq = ps_qk.tile([128, QK_CHUNK], F32, name="pqk", tag="pqk")
                    for k in range(3):
                        xTv = xT[k][:].rearrange("f (p c) -> f p c", c=112)
                        nc.tensor.matmul(
                            pq[:, 0:w],
                            lhsT=qkw_sb[k][:, 128 * m:128 * (m + 1)],
                            rhs=xTv[:, pc:pc + np_, 0:98],
                            start=(k == 0), stop=(k == 2),
                        )
                    nc.vector.tensor_scalar(
                        out=qk[m][:, 98 * pc:98 * pc + w], in0=pq[:, 0:w],
                        scalar1=cqk_sb[m][:], scalar2=None,
                        op0=mybir.AluOpType.add,
                    )

            if debug_dump and g == 0:
                for k in range(3):
                    nc.sync.dma_start(out=xT_o[k], in_=xT[k][:])
                for m in range(6):
                    nc.sync.dma_start(out=qk_o[m], in_=qk[m][:])

            # ---- phase C: per window pair --------------------------------------
            for p in range(PAIRS_G):
                # v projection: window w01 -> psum rows 64*w01..+49 (col tiling)
                pv = ps_v.tile([128, D], F32, name="pv", tag="pv")
                for w01 in range(2):
                    c0 = 112 * p + 49 * w01
                    for k in range(3):
                        nc.tensor.matmul(
                            pv[64 * w01:64 * w01 + 49, :],
                            lhsT=xT[k][:, c0:c0 + 49],
                            rhs=vw_sb[k][:],
                            start=(k == 0), stop=(k == 2),
                        )
                av = av_rot[p % NROT]
                nc.vector.tensor_tensor(
                    out=av[0:113, :, 0:HD],
                    in0=pv[0:113, :].rearrange("p (h d) -> p h d", d=HD),
                    in1=cv_sb[0:113, :].rearrange("p (h d) -> p h d", d=HD),
                    op=mybir.AluOpType.add,
                )

                if debug_dump and g == 0:
                    nc.sync.dma_start(out=av_o[p], in_=av[:].rearrange("p h d -> p (h d)"))

                # scores S^T[k,q] per (window, head) + bias matmul + exp
                p_t = work.tile([128, NH, N], BF16, name="pt", tag="pt")
                for quad in range(3):
                    ps = ps_s.tile([128, 4, N], F32, name="ps", tag="ps")
                    for j in range(4):
                        h = 4 * quad + j
                        qt = qk[h // 4]
                        kt = qk[3 + h // 4]
                        hb = 32 * (h % 4)
                        for w01 in range(2):
                            c0 = 98 * p + 49 * w01
                            ob = 64 * w01
                            nc.tensor.matmul(
                                ps[ob:ob + 49, j, :],
                                lhsT=kt[hb:hb + 32, c0:c0 + 49],
                                rhs=qt[hb:hb + 32, c0:c0 + 49],
                                start=True, stop=False,
                                tile_position=(hb, ob),
                            )
                            nc.tensor.matmul(
                                ps[ob:ob + 49, j, :],
                                lhsT=bmm_sb[ob:ob + 49, N * h:N * (h + 1)],
                                rhs=idup_sb[ob:ob + 49, 0:49],
                                start=False, stop=True,
                            )
                    nc.scalar.activation(
                        out=p_t[0:113, 4 * quad:4 * quad + 4, :],
                        in_=ps[0:113, :, :],
                        func=mybir.ActivationFunctionType.Exp,
                    )

                if debug_dump and g == 0:
                    nc.sync.dma_start(out=pt_o[p], in_=p_t[:].rearrange("p h n -> p (h n)"))

                # P @ [V | 1]  (lhsT is p_t directly -- already [k, q])
                po = ps_o.tile([128, NH, HD + 1], F32, name="po", tag="po")
                for h in range(NH):
                    for w01 in range(2):
                        ob = 64 * w01
                        nc.tensor.matmul(
                            po[ob:ob + 49, h, :],
                            lhsT=p_t[ob:ob + 49, h, :],
                            rhs=av[ob:ob + 49, h, :],
                            start=True, stop=True,
                        )
                rec = small.tile([128, NH], F32, name="rec", tag="rec")
                nc.vector.reciprocal(out=rec[0:113, :], in_=po[0:113, :, HD])
                at_sb = work.tile([128, D], BF16, name="at", tag="at")
                rec_sl = rec[0:113, :]
                rec_b = bass.AP(
                    tensor=rec_sl.tensor,
                    offset=rec_sl.offset,
                    ap=[*rec_sl.ap, [0, HD]],
                )
                nc.vector.tensor_tensor(
                    out=at_sb[0:113, :].rearrange("p (h d) -> p h d", d=HD),
                    in0=po[0:113, :, 0:HD],
                    in1=rec_b,
                    op=mybir.AluOpType.mult,
                )

                # transpose attention out to feature-major, proj, bias, store
                at_T = work.tile([128, 3 * 128], BF16, name="atT", tag="atT")
                for k in range(3):
                    nc.sync.dma_start(
                        out=at_T[:, 128 * k:128 * (k + 1)],
                        in_=at_sb[0:128, 128 * k:128 * (k + 1)],
                        transpose=True,
                    )
                pp = ps_p.tile([128, D], F32, name="pp", tag="pp")
                for w01 in range(2):
                    ob = 64 * w01
                    for k in range(3):
                        nc.tensor.matmul(
                            pp[ob:ob + 49, :],
                            lhsT=at_T[:, 128 * k + ob:128 * k + ob + 49],
                            rhs=pw_sb[k][:],
                            start=(k == 0), stop=(k == 2),
                        )
                y_sb = work.tile([128, D], F32, name="y", tag="y")
                nc.vector.tensor_tensor(
                    out=y_sb[0:113, :], in0=pp[0:113, :], in1=pb_sb[0:113, :],
                    op=mybir.AluOpType.add,
                )
                r0 = T0 + 98 * p
                nc.sync.dma_start(out=y_d[r0:r0 + 49, :], in_=y_sb[0:49, :])
                nc.sync.dma_start(out=y_d[r0 + 49:r0 + 98, :], in_=y_sb[64:113, :])
        if rep_ctx is not None:
            rep_ctx.__exit__(None, None, None)

    nc.compile()
    return nc


_NC_CACHE = {}


def _get_program():
    if "nc" not in _NC_CACHE:
        _NC_CACHE["nc"] = build_program()
    return _NC_CACHE["nc"]


def _window_order(xf):
    # [BT, H, W, D] -> [BT*nW*N, D] in window-raster order
    BT = xf.shape[0]
    x6 = xf.reshape(BT, H // WSZ, WSZ, W // WSZ, WSZ, D)
    return np.ascontiguousarray(x6.transpose(0, 1, 3, 2, 4, 5)).reshape(-1, D)


def _window_unorder(yw):
    BT = B * T
    y6 = yw.reshape(BT, H // WSZ, W // WSZ, WSZ, WSZ, D)
    return np.ascontiguousarray(y6.transpose(0, 1, 3, 2, 4, 5)).reshape(BT, H, W, D)


def prepare_inputs(x, ln_g, ln_b, qkv_w, qkv_b, proj_w, proj_b, rel_bias_table):
    x = np.asarray(x, np.float32)
    ln_g = np.asarray(ln_g, np.float32)
    ln_b = np.asarray(ln_b, np.float32)
    qkv_w = np.asarray(qkv_w, np.float32)
    qkv_b = np.asarray(qkv_b, np.float32)
    proj_w = np.asarray(proj_w, np.float32)
    proj_b = np.asarray(proj_b, np.float32)
    rel_bias_table = np.asarray(rel_bias_table, np.float32)

    scale = HD ** -0.5
    wq = qkv_w[:, :D] * ln_g[:, None] * scale
    wk = qkv_w[:, D:2 * D] * ln_g[:, None]
    wv = qkv_w[:, 2 * D:] * ln_g[:, None]
    cq = (ln_b @ qkv_w[:, :D] + qkv_b[:D]) * scale
    ck = ln_b @ qkv_w[:, D:2 * D] + qkv_b[D:2 * D]
    cv = ln_b @ qkv_w[:, 2 * D:] + qkv_b[2 * D:]

    qkw = np.concatenate([wq, wk], axis=1).astype(ml_dtypes.bfloat16)
    cqk = np.concatenate([cq, ck]).astype(np.float32)

    idx = _rel_index(WSZ)
    bias = rel_bias_table[idx.reshape(-1)].reshape(N, N, NH)  # [q, k, h]
    bmm = np.zeros((128, NH * N), np.float32)
    for h in range(NH):
        bmm[0:49, N * h:N * (h + 1)] = bias[:, :, h]
        bmm[64:113, N * h:N * (h + 1)] = bias[:, :, h]

    idup = np.zeros((128, 64), np.float32)
    idup[0:49, 0:49] = np.eye(49)
    idup[64:113, 0:49] = np.eye(49)

    xw = _window_order(x.reshape(B * T, H, W, D))

    common = {
        "qkw": qkw,
        "vw": wv.astype(ml_dtypes.bfloat16),
        "pw": proj_w.astype(ml_dtypes.bfloat16),
        "cqk": cqk,
        "cv": cv.astype(np.float32),
        "pb": proj_b.astype(np.float32),
        "bmm": bmm.astype(ml_dtypes.bfloat16),
        "idup": idup.astype(ml_dtypes.bfloat16),
    }
    in_maps = []
    for c in range(N_CORES):
        m = dict(common)
        m["x"] = np.ascontiguousarray(xw[TOK_CORE * c:TOK_CORE * (c + 1)])
        in_maps.append(m)
    return in_maps


def kernel(x, ln_g, ln_b, qkv_w, qkv_b, proj_w, proj_b, rel_bias_table):
    nc = _get_program()
    in_maps = prepare_inputs(x, ln_g, ln_b, qkv_w, qkv_b, proj_w, proj_b, rel_bias_table)
    res = run_bass_kernel_spmd(nc, in_maps, core_ids=list(range(N_CORES)))
    yw = np.concatenate([res.results[c]["y"] for c in range(N_CORES)], axis=0)
    out = _window_unorder(yw).reshape(B, T, H, W, D)
    return out.astype(np.float32)

